# revision 1
# baseline (speedup 1.0000x reference)
"""Single-head causal attention (B=8, T=2048, C=384, H=64) on 8 NeuronCores.

Data-parallel over batch: core b computes attention for batch element b.
Per-core pipeline (all matmuls bf16, fp32 PSUM accumulate):
  - host pre-transposes x -> xT [C, T] and casts to bf16
  - qT/kT = W{q,k}.T @ xT           (PE, contract C in 3 chunks of 128)
  - v     = x @ Wv                  (PE, natural [S, H] layout, + ones column)
  - ST    = kT_block.T @ qT         (PE, scores transposed: [s, t] layout)
  - PT    = exp(ST / sqrt(C))       (ACT, psum->sbuf, bf16 out; no max-sub
                                     needed: |S/sqrt(C)| < ~1 for this data)
  - causal: only lower-triangle blocks computed; diagonal 128x128 block
    masked multiplicatively after exp
  - out_i = sum_j PT_j[:,i].T @ [v_j | 1]  (PE, accumulated in PSUM; the ones
    column yields the softmax denominator in col 64)
  - out   = out[:, :64] * (1 / out[:, 64]) (DVE), DMA to DRAM f32
"""

import math
import os

import numpy as np
import ml_dtypes

import concourse.bass as bass
import concourse.tile as tile
from concourse import bacc, mybir
from concourse.bass import ds, ts
from concourse.bass_utils import run_bass_kernel_spmd

F32 = mybir.dt.float32
BF16 = mybir.dt.bfloat16

B, T, C, H = 8, 2048, 384, 64
P = 128
NT = T // P          # 16 t-tiles (query blocks)
NCC = C // P         # 3 contraction chunks
SCALE = 1.0 / math.sqrt(float(C))

# stash of the last run's results (test.py reads exec_time_ns from here)
LAST_RESULT = None
_PROGRAM = None


def _emit(tc: tile.TileContext, xT_d, wq_d, wk_d, wv_d, mask_d, out_d, ctx):
    nc = tc.nc
    Exp = mybir.ActivationFunctionType.Exp

    const = ctx.enter_context(tc.tile_pool(name="const", bufs=1))
    big = ctx.enter_context(tc.tile_pool(name="big", bufs=1))
    outp = ctx.enter_context(tc.tile_pool(name="outp", bufs=2))
    ps = ctx.enter_context(tc.tile_pool(name="ps", bufs=1, space="PSUM"))

    # ---- input DMAs -------------------------------------------------------
    wq_sb = const.tile([P, NCC, H], BF16, tag="wq")
    nc.sync.dma_start(wq_sb[:], wq_d.rearrange("(c p) h -> p c h", p=P))
    wk_sb = const.tile([P, NCC, H], BF16, tag="wk")
    nc.sync.dma_start(wk_sb[:], wk_d.rearrange("(c p) h -> p c h", p=P))
    wv_sb = const.tile([P, NCC, H], BF16, tag="wv")
    nc.sync.dma_start(wv_sb[:], wv_d.rearrange("(c p) h -> p c h", p=P))
    mask_sb = const.tile([P, P], BF16, tag="mask")
    nc.sync.dma_start(mask_sb[:], mask_d[:])

    xT = []
    for c in range(NCC):
        t = big.tile([P, T], BF16, tag=f"xT{c}", name=f"xT{c}")
        nc.sync.dma_start(t[:], xT_d[ts(c, P), :])
        xT.append(t)

    # ---- q/k projection: qk_sb[:, 0, :] = qT, qk_sb[:, 1, :] = kT ---------
    # qT[h, t] = sum_c Wq[c, h] * xT[c, t]   (only partitions 0..63 used)
    qk_sb = big.tile([H, 2, T], BF16, tag="qk")
    for t4 in range(4):
        pqk = ps.tile([P, 1024], F32, tag="st", bufs=3, name=f"pqk{t4}")
        for c in range(NCC):
            nc.tensor.matmul(
                pqk[0:H, 0:512], wq_sb[:, c, :], xT[c][:, ts(t4, 512)],
                start=(c == 0), stop=(c == NCC - 1),
            )
        for c in range(NCC):
            nc.tensor.matmul(
                pqk[0:H, 512:1024], wk_sb[:, c, :], xT[c][:, ts(t4, 512)],
                start=(c == 0), stop=(c == NCC - 1),
            )
        # one copy moves both q and k halves (cast f32 -> bf16)
        nc.vector.tensor_copy(
            qk_sb[:, :, ts(t4, 512)],
            pqk[0:H, :].rearrange("p (k n) -> p k n", k=2),
        )
    qT = qk_sb[:, 0, :]
    kT = qk_sb[:, 1, :]

    # ---- score pass S(j): ST psum -> exp -> PT_j (sbuf, bf16) -------------
    pt_tiles = {}

    def emit_S(j):
        t0 = P * j                   # first t column computed for this block
        W = T - t0
        ktj = kT[:, ds(P * j, P)]
        pt = big.tile([P, W], BF16, tag=f"pt{j}", name=f"pt{j}")
        pt_tiles[j] = (pt, t0)
        for off in range(0, W, 1024):
            w = min(1024, W - off)
            st = ps.tile([P, 1024], F32, tag="st", bufs=3, name=f"st{j}_{off}")
            for o2 in range(0, w, 512):
                n2 = min(512, w - o2)
                nc.tensor.matmul(
                    st[:, ds(o2, n2)], ktj, qT[:, ds(t0 + off + o2, n2)],
                    start=True, stop=True,
                )
            nc.scalar.activation(pt[:, ds(off, w)], st[:, 0:w], Exp, scale=SCALE)
        # mask the diagonal block (at offset 0): keep s <= t only
        nc.vector.tensor_mul(pt[:, 0:P], pt[:, 0:P], mask_sb[:])

    emit_S(0)
    emit_S(1)

    # ---- v projection (+ ones column for the softmax denominator) --------
    v_sb = big.tile([P, NT, 66], BF16, tag="v")
    for pk in range(2):
        pv = ps.tile([P, 1024], F32, tag="st", bufs=3, name=f"pv{pk}")
        for jj in range(8):
            j = 8 * pk + jj
            for c in range(NCC):
                nc.tensor.matmul(
                    pv[:, ts(jj, H)], xT[c][:, ds(P * j, P)], wv_sb[:, c, :],
                    start=(c == 0), stop=(c == NCC - 1),
                )
        nc.vector.tensor_copy(
            v_sb[:, ds(8 * pk, 8), 0:H],
            pv[:, 0:512].rearrange("p (j h) -> p j h", h=H),
        )
    nc.vector.memset(v_sb[:, :, H:65], 1.0)

    # ---- output pass O(i): PV accumulate, normalize, store ----------------
    out_v = out_d.rearrange("(g i p) h -> g p i h", p=P, i=4)
    ob = None
    for i in range(NT):
        if i + 2 < NT:
            emit_S(i + 2)
        if i % 4 == 0:
            ob = outp.tile([P, 4, H], F32, tag="osb", bufs=2, name=f"ob{i // 4}")
        oa = ps.tile([P, 72], F32, tag="oacc", bufs=2, name=f"oacc{i}")
        for j in range(i + 1):
            pt, t0 = pt_tiles[j]
            nc.tensor.matmul(
                oa[:, 0:65], pt[:, ds(P * i - t0, P)], v_sb[:, j, 0:65],
                start=(j == 0), stop=(j == i),
            )
        r = outp.tile([P, 1], F32, tag="recip", bufs=2, name=f"r{i}")
        nc.vector.reciprocal(r[:], oa[:, 64:65])
        nc.vector.tensor_scalar_mul(ob[:, i % 4, :], oa[:, 0:H], r[:])
        if i % 4 == 3:
            nc.sync.dma_start(out_v[i // 4], ob[:])


def _build_program():
    nc = bacc.Bacc("TRN2", target_bir_lowering=False, debug=False, num_devices=B)
    xT_d = nc.dram_tensor("xT", [C, T], BF16, kind="ExternalInput").ap()
    wq_d = nc.dram_tensor("wq", [C, H], BF16, kind="ExternalInput").ap()
    wk_d = nc.dram_tensor("wk", [C, H], BF16, kind="ExternalInput").ap()
    wv_d = nc.dram_tensor("wv", [C, H], BF16, kind="ExternalInput").ap()
    mask_d = nc.dram_tensor("mask", [P, P], BF16, kind="ExternalInput").ap()
    out_d = nc.dram_tensor("out", [T, H], F32, kind="ExternalOutput").ap()
    from contextlib import ExitStack

    with tile.TileContext(nc) as tc:
        with ExitStack() as ctx:
            _emit(tc, xT_d, wq_d, wk_d, wv_d, mask_d, out_d, ctx)
    nc.compile()
    return nc


def kernel(x, Wq, Wk, Wv):
    global LAST_RESULT, _PROGRAM
    assert x.shape == (B, T, C), x.shape
    if _PROGRAM is None:
        _PROGRAM = _build_program()
    nc = _PROGRAM

    bf = ml_dtypes.bfloat16
    xT = np.ascontiguousarray(np.transpose(x, (0, 2, 1))).astype(bf)
    wq = np.ascontiguousarray(Wq).astype(bf)
    wk = np.ascontiguousarray(Wk).astype(bf)
    wv = np.ascontiguousarray(Wv).astype(bf)
    # mask[s, t] = 1 where s <= t (transposed-causal, diagonal 128x128 block)
    mask = np.triu(np.ones((P, P), dtype=np.float32)).astype(bf)

    in_maps = [
        {"xT": xT[b], "wq": wq, "wk": wk, "wv": wv, "mask": mask}
        for b in range(B)
    ]
    trace = bool(int(os.environ.get("KERNEL_TRACE", "0")))
    kw = {}
    td = os.environ.get("KERNEL_TRACE_DIR")
    if td:
        kw["tmpdir"] = td
    LAST_RESULT = run_bass_kernel_spmd(
        nc, in_maps, list(range(B)), trace=trace, **kw
    )
    out = np.stack([LAST_RESULT.results[b]["out"] for b in range(B)], axis=0)
    return out.astype(np.float32)

